# revision 1
# baseline (speedup 1.0000x reference)
"""GIN message-passing network on 8 Trainium2 NeuronCores.

Strategy (graph/data-parallel per sharding hint):
- 512 graphs -> 64 graphs per core; each core owns the contiguous node range of
  its graphs (batch is sorted). Edges are assigned to the core owning dst.
- Per layer (one SPMD launch, same compiled program 3x):
    gather x[src] rows via indirect DMA (one 128-row chunk per call, int32
    offsets), convert to bf16, scatter-add via PE matmuls against one-hot dst
    masks (built on DVE with iota/is_equal), accumulating agg^T in PSUM per
    128-node window; agg += x_local; 64x64 MLP on PE; transpose back; global
    mean-pool partials and the linear head on-device.
- Host (numpy) only reorders/shards data between launches.
"""

import math
import numpy as np

import concourse.bacc as bacc
import concourse.bass as bass
import concourse.mybir as mybir
import concourse.tile as tile
from concourse.bass_utils import run_bass_kernel_spmd

F32 = mybir.dt.float32
BF16 = mybir.dt.bfloat16
I32 = mybir.dt.int32
AF = mybir.ActivationFunctionType

D = 64          # feature dim
GB = 32         # gather batch, in 128-edge chunks
MB = 8          # mask batch, in chunks


def _bcast_inner(ap, n):
    """[P, L] AP -> [P, L, n] with stride-0 inner (free-dim broadcast)."""
    return bass.AP(ap.tensor, ap.offset, list(ap.ap) + [[0, n]])


def build_program(NP, WC, NR, G64=64, ones_x=False):
    """One SPMD per-core layer program. NP: padded nodes/core (mult of 512).
    WC: chunks per 128-node window. NR: gather table rows."""
    W = NP // 128
    CT = W * WC
    NT = NP // 512  # MLP tiles

    nc = bacc.Bacc("TRN2", target_bir_lowering=False, debug=False)
    x_d = nc.dram_tensor("x", [NR, D], F32, kind="ExternalInput")
    xlt_d = nc.dram_tensor("xlt", [D, NP], F32, kind="ExternalInput")
    gi_d = nc.dram_tensor("gi", [128, CT], I32, kind="ExternalInput")
    dr_d = nc.dram_tensor("dr", [128, CT], F32, kind="ExternalInput")
    gl_d = nc.dram_tensor("gl", [128, W], F32, kind="ExternalInput")
    wa_d = nc.dram_tensor("wa", [D, D], F32, kind="ExternalInput")
    ba_d = nc.dram_tensor("ba", [D, 1], F32, kind="ExternalInput")
    wb_d = nc.dram_tensor("wb", [D, D], F32, kind="ExternalInput")
    bb_d = nc.dram_tensor("bb", [D, 1], F32, kind="ExternalInput")
    wlb_d = nc.dram_tensor("wlb", [G64, D], F32, kind="ExternalInput")
    blc_d = nc.dram_tensor("blc", [G64, 1], F32, kind="ExternalInput")
    io8_d = nc.dram_tensor("io8", [128, MB, 128], F32, kind="ExternalInput")
    iog_d = nc.dram_tensor("iog", [128, G64], F32, kind="ExternalInput")
    id_d = nc.dram_tensor("idm", [128, 128], F32, kind="ExternalInput")
    xn_d = nc.dram_tensor("xn", [NP, D], F32, kind="ExternalOutput")
    hd_d = nc.dram_tensor("hd", [G64, 1], F32, kind="ExternalOutput")

    with tile.TileContext(nc) as tc:
        with (
            tc.tile_pool(name="st", bufs=1) as st,          # static/resident
            tc.tile_pool(name="yp", bufs=4) as yp,          # gather streams
            tc.tile_pool(name="mp", bufs=4) as mp,          # mask batches
            tc.tile_pool(name="wp", bufs=2) as wp,          # mlp work
            tc.tile_pool(name="pp", bufs=1, space="PSUM") as pp,
            tc.tile_pool(name="ptp", bufs=2, space="PSUM") as ptp,
            tc.tile_pool(name="pagg", bufs=2, space="PSUM") as pagg,
            tc.tile_pool(name="ppool", bufs=1, space="PSUM") as ppool,
        ):
            # ---- static loads ----
            gi_t = st.tile([128, CT], I32)
            dr_t = st.tile([128, CT], F32)
            gl_t = st.tile([128, W], F32)
            xlt_t = st.tile([D, NP], F32)
            wa_t = st.tile([D, D], F32)
            ba_t = st.tile([D, 1], F32)
            wb_t = st.tile([D, D], F32)
            bb_t = st.tile([D, 1], F32)
            wlb_t = st.tile([G64, D], F32)
            blc_t = st.tile([G64, 1], F32)
            iota8 = st.tile([128, MB, 128], F32)
            iotg = st.tile([128, G64], F32)
            id_t = st.tile([128, 128], F32)
            for dst_t, src_d in ((gi_t, gi_d), (dr_t, dr_d), (gl_t, gl_d),
                                 (xlt_t, xlt_d), (wa_t, wa_d), (ba_t, ba_d),
                                 (wb_t, wb_d), (bb_t, bb_d), (wlb_t, wlb_d),
                                 (blc_t, blc_d), (iota8, io8_d), (iotg, iog_d),
                                 (id_t, id_d)):
                nc.sync.dma_start(dst_t[:], src_d[:])

            ones_y = st.tile([128, D], BF16)
            nc.vector.memset(ones_y[:], 1.0)
            aggT = st.tile([D, NP], F32)
            xT = st.tile([D, NP], F32)
            xrow = st.tile([128, W, D + 1], F32)
            nc.vector.memset(xrow[:, :, D : D + 1], 1.0)

            # ---- streaming helpers ----
            ytiles = {}

            def get_y(c):
                """bf16 Y chunk [128, 64] for global chunk c."""
                b = c // GB
                if b not in ytiles:
                    take = min(GB, CT - b * GB)
                    yf = yp.tile([128, GB, D], F32, tag="y")
                    for j in range(take):
                        nc.gpsimd.indirect_dma_start(
                            out=yf[:, j, :], out_offset=None, in_=x_d[:],
                            in_offset=bass.IndirectOffsetOnAxis(
                                ap=gi_t[:, b * GB + j : b * GB + j + 1], axis=0),
                        )
                    yb = yp.tile([128, GB, D], BF16, tag="yb")
                    nc.vector.tensor_copy(out=yb[:, :take, :], in_=yf[:, :take, :])
                    ytiles[b] = yb
                return ytiles[b][:, c - b * GB, :]

            mtiles = {}

            def get_mask(c):
                """bf16 one-hot mask [128e, 128n] for global chunk c."""
                m = c // MB
                if m not in mtiles:
                    take = min(MB, CT - m * MB)
                    mk = mp.tile([128, MB, 128], BF16, tag="m")
                    nc.vector.tensor_tensor(
                        out=mk[:, :take, :],
                        in0=iota8[:, :take, :],
                        in1=_bcast_inner(dr_t[:, m * MB : m * MB + take], 128),
                        op=mybir.AluOpType.is_equal,
                    )
                    mtiles[m] = mk
                return mtiles[m][:, c - m * MB, :]

            # ---- scatter: agg^T per window ----
            for w in range(W):
                ps = pagg.tile([D, 128], F32, space="PSUM", tag="agg")
                for j in range(WC):
                    c = w * WC + j
                    lhs = ones_y[:] if ones_x else get_y(c)
                    nc.tensor.matmul(out=ps[:], lhsT=lhs, rhs=get_mask(c),
                                     start=(j == 0), stop=(j == WC - 1))
                nc.vector.tensor_tensor(
                    out=aggT[:, w * 128 : (w + 1) * 128],
                    in0=ps[:],
                    in1=xlt_t[:, w * 128 : (w + 1) * 128],
                    op=mybir.AluOpType.add,
                )

            # ---- MLP: xT = relu(wb^T relu(wa^T aggT + ba) + bb) ----
            for t in range(NT):
                sl = slice(t * 512, (t + 1) * 512)
                hp = pp.tile([D, 512], F32, space="PSUM", tag="hps")
                nc.tensor.matmul(out=hp[:], lhsT=wa_t[:], rhs=aggT[:, sl],
                                 start=True, stop=True)
                hT = wp.tile([D, 512], F32, tag="hT")
                nc.scalar.activation(out=hT[:], in_=hp[:], func=AF.Relu,
                                     bias=ba_t[:, 0:1], scale=1.0)
                xp = pp.tile([D, 512], F32, space="PSUM", tag="xps")
                nc.tensor.matmul(out=xp[:], lhsT=wb_t[:], rhs=hT[:],
                                 start=True, stop=True)
                nc.scalar.activation(out=xT[:, sl], in_=xp[:], func=AF.Relu,
                                     bias=bb_t[:, 0:1], scale=1.0)

            # ---- transpose back + pooling ----
            pool_ps = ppool.tile([G64, D + 1], F32, space="PSUM", tag="pool")
            for w in range(W):
                tp = ptp.tile([128, D], F32, space="PSUM", tag="tp")
                nc.tensor.transpose(out=tp[:], in_=xT[:, w * 128 : (w + 1) * 128],
                                    identity=id_t[:64, :64])
                nc.scalar.activation(out=xrow[:, w, :D], in_=tp[:], func=AF.Copy)
                mpo = mp.tile([128, G64], F32, tag="mpool")
                nc.vector.tensor_tensor(
                    out=mpo[:], in0=iotg[:],
                    in1=gl_t[:, w : w + 1].to_broadcast([128, G64]),
                    op=mybir.AluOpType.is_equal,
                )
                nc.tensor.matmul(out=pool_ps[:], lhsT=mpo[:], rhs=xrow[:, w, :],
                                 start=(w == 0), stop=(w == W - 1))
            nc.sync.dma_start(
                xn_d[:].rearrange("(c p) f -> p c f", p=128), xrow[:, :, :D]
            )

            # ---- head: ((sums/cnt) * wl).sum(f) + bl ----
            pool_sb = wp.tile([G64, D + 1], F32, tag="poolsb")
            nc.vector.tensor_copy(out=pool_sb[:], in_=pool_ps[:])
            cntm = wp.tile([G64, 1], F32, tag="cntm")
            nc.vector.tensor_scalar_max(cntm[:], pool_sb[:, D : D + 1], 1.0)
            rcnt = wp.tile([G64, 1], F32, tag="rcnt")
            nc.vector.reciprocal(rcnt[:], cntm[:])
            pooled = wp.tile([G64, D], F32, tag="pooled")
            nc.vector.tensor_scalar(out=pooled[:], in0=pool_sb[:, :D],
                                    scalar1=rcnt[:, 0:1], scalar2=None,
                                    op0=mybir.AluOpType.mult)
            prod = wp.tile([G64, D], F32, tag="prod")
            nc.vector.tensor_tensor(out=prod[:], in0=pooled[:], in1=wlb_t[:],
                                    op=mybir.AluOpType.mult)
            hsum = wp.tile([G64, 1], F32, tag="hsum")
            nc.vector.tensor_reduce(out=hsum[:], in_=prod[:],
                                    axis=mybir.AxisListType.X,
                                    op=mybir.AluOpType.add)
            hout = wp.tile([G64, 1], F32, tag="hout")
            nc.vector.tensor_tensor(out=hout[:], in0=hsum[:], in1=blc_t[:],
                                    op=mybir.AluOpType.add)
            nc.sync.dma_start(hd_d[:], hout[:])

    nc.compile()
    return nc


def preprocess(edge_index, batch, n_nodes, n_graphs, C=8):
    src = np.asarray(edge_index[0], np.int64)
    dst = np.asarray(edge_index[1], np.int64)
    batch = np.asarray(batch, np.int64)
    E = src.shape[0]
    GPC = n_graphs // C
    node_start = np.searchsorted(batch, np.arange(C) * GPC)
    node_end = np.append(node_start[1:], n_nodes)
    counts = node_end - node_start
    NP = max(512, int(math.ceil(counts.max() / 512.0)) * 512)
    W = NP // 128

    core_e = np.searchsorted(node_start, dst, side="right") - 1
    dst_local = dst - node_start[core_e]
    win = dst_local >> 7

    key = core_e * W + win
    cnt = np.bincount(key, minlength=C * W)
    WC = max(1, int(math.ceil(cnt.max() / 128.0)))
    CT = W * WC

    order = np.argsort(key, kind="stable")
    key_s = key[order]
    grp_cnt = np.bincount(key_s, minlength=C * W)
    grp_start = np.concatenate(([0], np.cumsum(grp_cnt)))
    off = np.arange(E) - grp_start[key_s]

    gi = np.zeros((C, CT * 128), np.int32)
    dr = np.full((C, CT * 128), -1.0, np.float32)
    co = core_e[order]
    wo = win[order]
    slot = wo * (WC * 128) + off
    gi[co, slot] = src[order].astype(np.int32)
    dr[co, slot] = (dst_local - win * 128).astype(np.float32)[order]

    gi_c = gi.reshape(C, CT, 128).transpose(0, 2, 1).copy()
    dr_c = dr.reshape(C, CT, 128).transpose(0, 2, 1).copy()

    gl = np.full((C, NP), -1.0, np.float32)
    for k in range(C):
        gl[k, : counts[k]] = batch[node_start[k] : node_end[k]] - k * GPC
    gl_c = gl.reshape(C, W, 128).transpose(0, 2, 1).copy()

    return dict(
        C=C, GPC=GPC, NP=NP, W=W, WC=WC,
        node_start=node_start, node_end=node_end, counts=counts,
        gi=gi_c, dr=dr_c, gl=gl_c,
    )


_PROG_CACHE = {}


def _get_program(NP, WC, NR, ones_x=False):
    key = (NP, WC, NR, ones_x)
    if key not in _PROG_CACHE:
        _PROG_CACHE[key] = build_program(NP, WC, NR, ones_x=ones_x)
    return _PROG_CACHE[key]


def kernel(**inputs):
    feats = np.asarray(inputs["features"], np.float32)
    ei = np.asarray(inputs["edge_index"])
    batch = np.asarray(inputs["batch"])
    N, _ = feats.shape
    G = 512
    C = 8

    pre = preprocess(ei, batch, N, G, C=C)
    NP, W, MBc = pre["NP"], pre["W"], MB
    NR = int(math.ceil(N / 128.0)) * 128
    nc = _get_program(NP, pre["WC"], NR)

    layers = [
        (inputs["w1a"], inputs["b1a"], inputs["w1b"], inputs["b1b"]),
        (inputs["w2a"], inputs["b2a"], inputs["w2b"], inputs["b2b"]),
        (inputs["w3a"], inputs["b3a"], inputs["w3b"], inputs["b3b"]),
    ]
    wl = np.asarray(inputs["wl"], np.float32)
    bl = np.asarray(inputs["bl"], np.float32)
    wlb = np.tile(wl[:, 0][None, :], (64, 1)).astype(np.float32)
    blc = np.full((64, 1), float(bl[0]), np.float32)
    io8 = np.broadcast_to(
        np.arange(128, dtype=np.float32)[None, None, :], (128, MBc, 128)
    ).copy()
    iog = np.broadcast_to(
        np.arange(64, dtype=np.float32)[None, :], (128, 64)
    ).copy()
    idm = np.eye(128, dtype=np.float32)

    x = feats
    head = None
    ns, ne, cnts = pre["node_start"], pre["node_end"], pre["counts"]
    ones_first = bool(np.all(feats == 1.0))
    for li, (wa, ba, wb, bb) in enumerate(layers):
        use_ones = ones_first and li == 0
        if use_ones:
            try:
                nc_l = _get_program(NP, pre["WC"], NR, ones_x=True)
            except Exception:
                nc_l = nc
        else:
            nc_l = nc
        xpad = np.zeros((NR, D), np.float32)
        xpad[:N] = x
        in_maps = []
        for k in range(C):
            xlt = np.zeros((D, NP), np.float32)
            xlt[:, : cnts[k]] = x[ns[k] : ne[k]].T
            in_maps.append({
                "x": xpad, "xlt": xlt,
                "gi": pre["gi"][k], "dr": pre["dr"][k], "gl": pre["gl"][k],
                "wa": np.asarray(wa, np.float32),
                "ba": np.asarray(ba, np.float32).reshape(D, 1),
                "wb": np.asarray(wb, np.float32),
                "bb": np.asarray(bb, np.float32).reshape(D, 1),
                "wlb": wlb, "blc": blc,
                "io8": io8, "iog": iog, "idm": idm,
            })
        res = run_bass_kernel_spmd(nc_l, in_maps, core_ids=list(range(C)))
        xn = np.empty((N, D), np.float32)
        for k in range(C):
            xn[ns[k] : ne[k]] = res.results[k]["xn"][: cnts[k]]
        x = xn
        head = np.concatenate(
            [res.results[k]["hd"][: pre["GPC"]] for k in range(C)], axis=0
        )
    return head.astype(np.float32)

